# revision 12
# baseline (speedup 1.0000x reference)
"""DMPNN message-passing kernel for 8 Trainium2 NeuronCores (Bass/Tile), v2.

Strategy (edge/data parallel):
  - Edge pairs (2k, 2k+1) sharded across 8 cores; each core splits pairs into
    arrays hE/hO so rev(e) is the same row of the sibling array.
  - Everything fp16.  relu is positively homogeneous and the reference biases
    enter linearly, so h is computed at scale s=2^-12 (folded into Wi/bi/bu
    offline, un-folded via Wf) to stay in fp16 range.
  - The aggregate node table lives in SBUF (packed 256B tokens, partition =
    token & 127).  Gathers are SBUF-source dma_gather(transpose=True) which
    produce feature-major tiles directly: no HBM random reads, no PE
    transposes on the gather side.
  - segment_sum = fp16 dma_scatter_add into an HBM table (windowed so each
    instruction has distinct rows), then a 2-half fp16 AllReduce with Shared
    outputs, overlapped with compute of the independent half, then a bulk
    partition-contiguous reload of the SBUF table.
"""
import os
import sys

sys.path.insert(0, "/opt/trn_rl_repo")

import numpy as np

N_CORES = 8
D = 128
DE = 32
STEPS = 4
SLAB = 512
GROUP = 2048
UNROLL = 4
HB = 32256            # LO real-node boundary (63*512)
T_LO = 256            # free-dim token slots per partition, LO table
LO_CAP = T_LO * 128   # 32768
LO_TRASH = HB         # row-in-half used for padded slots
HI_BASE = LO_CAP
T_HI = 140
HI_CAP = T_HI * 128   # 17920
NTAB = HI_BASE + HI_CAP   # 50688
SCALE = np.float32(2.0 ** -12)


def _ceil(x, m):
    return (x + m - 1) // m * m


def _tok(r, t):
    """node-row-in-half -> SBUF token id (partition-contiguous bulk load)."""
    return (r % t) * 128 + r // t


def _window_assign(s, d, group, node_max, max_win=192):
    """Assign each pair to a window of size `group` such that within every
    window all d values are distinct and all s values are distinct (the
    dma_scatter_add engine-race constraint).  Greedy rounds, vectorized."""
    n = s.size
    win = np.full(n, -1, np.int32)
    used_s = np.zeros((node_max, max_win), bool)
    used_d = np.zeros((node_max, max_win), bool)
    full = np.zeros(max_win, bool)
    cnt = np.zeros(max_win, np.int64)
    rem = np.arange(n)
    while rem.size:
        free = ~(used_s[s[rem]] | used_d[d[rem]] | full[None, :])
        assert free.any(axis=1).all(), "window assigner ran out of windows"
        w = np.argmax(free, axis=1).astype(np.int64)
        order = np.lexsort((rem, w))
        ws, rs = w[order], rem[order]
        ds_, ss_ = d[rs], s[rs]
        kd = ws * np.int64(node_max) + ds_
        ks = ws * np.int64(node_max) + ss_
        first_d = np.zeros(ws.size, bool)
        first_s = np.zeros(ws.size, bool)
        od = np.lexsort((np.arange(ws.size), kd))
        first_d[od[np.concatenate(([True], kd[od][1:] != kd[od][:-1]))]] = True
        os_ = np.lexsort((np.arange(ws.size), ks))
        first_s[os_[np.concatenate(([True], ks[os_][1:] != ks[os_][:-1]))]] = True
        uw, st, cts = np.unique(ws, return_index=True, return_counts=True)
        rank = np.arange(ws.size) - np.repeat(st, cts)
        ok = first_d & first_s & (rank < np.repeat(group - cnt[uw], cts))
        acc = rs[ok]
        wacc = ws[ok]
        win[acc] = wacc
        used_d[d[acc], wacc] = True
        used_s[s[acc], wacc] = True
        np.add.at(cnt, wacc, 1)
        full = cnt >= group
        rem = rem[win[rem] < 0]
    return win, (int(cnt.nonzero()[0].max()) + 1) if n else 0


def _prep(node_feature, edge_feature, edge_src, edge_dst,
          n_cores=N_CORES, group=GROUP, unroll=UNROLL,
          hb=HB, t_lo=T_LO, t_hi=T_HI):
    LO_CAP = t_lo * 128
    HI_BASE = LO_CAP
    HI_CAP = t_hi * 128
    NTAB = HI_BASE + HI_CAP
    HB_, T_LO_, T_HI_, LO_TRASH_ = hb, t_lo, t_hi, hb
    node_feature = np.asarray(node_feature, np.float32)
    edge_feature = np.asarray(edge_feature, np.float16)
    edge_src = np.asarray(edge_src)
    edge_dst = np.asarray(edge_dst)
    N = node_feature.shape[0]
    E = edge_src.shape[0]
    P = E // 2
    assert P % n_cores == 0
    per = P // n_cores
    assert N <= hb + HI_CAP - 1

    s_all = edge_src[0::2].astype(np.int64)
    d_all = edge_dst[0::2].astype(np.int64)
    efE_all = edge_feature[0::2]
    efO_all = edge_feature[1::2]

    HI_TRASH = N - hb      # row-in-half for HI trash

    # padded node-feature table, fp16, trash/pad rows zero
    nfp = np.zeros((NTAB, D), np.float16)
    nfp[0:hb] = node_feature[0:hb].astype(np.float16)
    nfp[HI_BASE:HI_BASE + (N - hb)] = node_feature[hb:N].astype(np.float16)

    cores = []
    nwin = np.zeros((n_cores, 4), np.int64)
    for c in range(n_cores):
        sl = slice(c * per, (c + 1) * per)
        sc, dc = s_all[sl], d_all[sl]
        a = (dc >= hb).astype(np.int64)
        b = (sc >= hb).astype(np.int64)
        seg = a * 2 + b
        per_seg = []
        for g in range(4):
            m = np.flatnonzero(seg == g)
            s_m, d_m = sc[m], dc[m]
            degd = np.bincount(d_m, minlength=N)
            degs = np.bincount(s_m, minlength=N)
            prio = np.argsort(-(degd[d_m] + degs[s_m]), kind="stable")
            win_p, nw = _window_assign(s_m[prio], d_m[prio], group, N)
            win = np.empty_like(win_p)
            win[prio] = win_p
            key = np.lexsort((s_m, win))
            per_seg.append((m[key], win[key], nw))
            nwin[c, g] = nw
        cores.append((sc, dc, efE_all[sl], efO_all[sl], per_seg))
    gchunk = group * unroll
    seg_nw = [int(_ceil(max(int(nwin[:, g].max()), 1) * group, gchunk)) // group
              for g in range(4)]
    seg_sz = [nw * group for nw in seg_nw]
    NP_ = int(sum(seg_sz))
    seg_start = [0, seg_sz[0], seg_sz[0] + seg_sz[1],
                 seg_sz[0] + seg_sz[1] + seg_sz[2]]

    def wrap16(v):
        t = v.astype(np.int16).reshape(-1, 16).T       # [16, n/16]
        return np.ascontiguousarray(np.tile(t, (8, 1)))  # [128, n/16]

    shards = []
    for c in range(n_cores):
        sc, dc, efE_c, efO_c, per_seg = cores[c]
        sRow = np.zeros(NP_, np.int64)   # scatter row within half b
        dRow = np.zeros(NP_, np.int64)   # scatter row within half a
        sTok = np.zeros(NP_, np.int64)   # gather token within half b
        dTok = np.zeros(NP_, np.int64)   # gather token within half a
        efE_p = np.zeros((NP_, DE), np.float16)
        efO_p = np.zeros((NP_, DE), np.float16)
        for g in range(4):
            a, b = g // 2, g % 2
            st = seg_start[g]
            t_a, t_b = (t_hi if a else t_lo), (t_hi if b else t_lo)
            tr_a = HI_TRASH if a else LO_TRASH_
            tr_b = HI_TRASH if b else LO_TRASH_
            sRow[st:st + seg_sz[g]] = tr_b
            dRow[st:st + seg_sz[g]] = tr_a
            sTok[st:st + seg_sz[g]] = _tok(tr_b, t_b)
            dTok[st:st + seg_sz[g]] = _tok(tr_a, t_a)
            order, wins, nw = per_seg[g]
            if order.size:
                counts = np.bincount(wins, minlength=nw)
                assert counts.max() <= group
                starts = np.concatenate(([0], np.cumsum(counts)))[:-1]
                rank = np.arange(order.size) - starts[wins]
                pos = st + wins * group + rank
                sr = sc[order] - hb * b
                dr = dc[order] - hb * a
                sRow[pos] = sr
                dRow[pos] = dr
                sTok[pos] = _tok(sr, t_b)
                dTok[pos] = _tok(dr, t_a)
                efE_p[pos] = efE_c[order]
                efO_p[pos] = efO_c[order]
        assert sRow.max() < 32768 and dRow.max() < 32768
        assert sTok.max() < 32768 and dTok.max() < 32768

        # idx stream: per group g: [sTok | dTok | dRow | sRow]
        gidx = np.empty((NP_ // group, 4, group), np.int64)
        for g in range(NP_ // group):
            sl = slice(g * group, (g + 1) * group)
            gidx[g, 0] = sTok[sl]
            gidx[g, 1] = dTok[sl]
            gidx[g, 2] = dRow[sl]
            gidx[g, 3] = sRow[sl]
        shards.append({
            "nfp": nfp,
            "efE": np.ascontiguousarray(efE_p.T),
            "efO": np.ascontiguousarray(efO_p.T),
            "gIdx": wrap16(gidx.reshape(-1)),
        })

    meta = dict(N=N, NP=NP_, seg_sz=seg_sz, seg_start=seg_start,
                n_cores=n_cores, group=group, unroll=unroll,
                HB=hb, T_LO=t_lo, T_HI=t_hi, LO_CAP=LO_CAP,
                HI_BASE=HI_BASE, HI_CAP=HI_CAP, NTAB=NTAB)
    return shards, meta


def _build(meta):
    import concourse.bass as bass
    import concourse.tile as tile
    from concourse import bacc, mybir

    f32 = mybir.dt.float32
    f16 = mybir.dt.float16
    i16 = mybir.dt.int16
    NP_ = meta["NP"]
    group = meta["group"]
    unroll = meta["unroll"]
    n_cores = meta["n_cores"]
    T_LO = meta["T_LO"]
    T_HI = meta["T_HI"]
    LO_CAP = meta["LO_CAP"]
    HI_BASE = meta["HI_BASE"]
    HI_CAP = meta["HI_CAP"]
    NTAB = meta["NTAB"]
    REPEAT = int(os.environ.get("KERNEL_REPEAT", "1"))

    nc = bacc.Bacc("TRN2", target_bir_lowering=False, debug=False,
                   enable_asserts=False, num_devices=n_cores,
                   num_swdge_queues=4)

    nfp_t = nc.dram_tensor("nfp", [NTAB, D], f16, kind="ExternalInput")
    efE_t = nc.dram_tensor("efE", [DE, NP_], f16, kind="ExternalInput")
    efO_t = nc.dram_tensor("efO", [DE, NP_], f16, kind="ExternalInput")
    gIdx_t = nc.dram_tensor("gIdx", [128, 4 * NP_ // 16], i16, kind="ExternalInput")
    WiA_t = nc.dram_tensor("WiA", [D, D], f16, kind="ExternalInput")
    WiB_t = nc.dram_tensor("WiB", [DE, D], f16, kind="ExternalInput")
    Wu_t = nc.dram_tensor("Wu", [D, D], f16, kind="ExternalInput")
    WfA_t = nc.dram_tensor("WfA", [D, D], f16, kind="ExternalInput")
    WfB_t = nc.dram_tensor("WfB", [D, D], f16, kind="ExternalInput")
    id_t = nc.dram_tensor("ident", [D, D], f16, kind="ExternalInput")
    bi_t = nc.dram_tensor("bi", [D, 1], f32, kind="ExternalInput")
    bu_t = nc.dram_tensor("bu", [D, 1], f32, kind="ExternalInput")
    bf4_t = nc.dram_tensor("bf4", [D, SLAB], f32, kind="ExternalInput")
    out_t = nc.dram_tensor("out", [NTAB, D], f32, kind="ExternalOutput")
    GP = group // 128
    GS = group // SLAB
    NB = SLAB // 128

    with tile.TileContext(nc) as tc:
        with (
            tc.tile_pool(name="const", bufs=1) as constp,
            tc.tile_pool(name="tabp", bufs=1) as tabp,
            tc.tile_pool(name="gip", bufs=3) as gip,
            tc.tile_pool(name="work", bufs=3) as work,
            tc.tile_pool(name="emb", bufs=2) as emb,
            tc.tile_pool(name="psA", bufs=2, space="PSUM") as psA,
            tc.tile_pool(name="psB", bufs=2, space="PSUM") as psB,
            tc.tile_pool(name="dram", bufs=1, space="DRAM") as dram,
        ):
            # ---- constants ----
            def cload(name, shape, dt_, src):
                t = constp.tile(shape, dt_, tag=name, name=name)
                nc.sync.dma_start(t[:], src)
                return t

            WiA = cload("WiA", [D, D], f16, WiA_t.ap())
            WiB = cload("WiB", [DE, D], f16, WiB_t.ap())
            Wu = cload("Wu", [D, D], f16, Wu_t.ap())
            WfA = cload("WfA", [D, D], f16, WfA_t.ap())
            WfB = cload("WfB", [D, D], f16, WfB_t.ap())
            id16 = cload("id16", [D, D], f16, id_t.ap())
            bi_sb = cload("bi", [D, 1], f32, bi_t.ap())
            bu_sb = cload("bu", [D, 1], f32, bu_t.ap())
            bf4_sb = cload("bf4", [D, SLAB], f32, bf4_t.ap())
            zero_sb = constp.tile([128, 2048], f16, tag="zero", name="zero")
            nc.vector.memset(zero_sb[:], 0.0)

            # ---- SBUF node tables ----
            tabLO = tabp.tile([128, T_LO * D], f16, tag="tabLO", name="tabLO")
            tabHI = tabp.tile([128, T_HI * D], f16, tag="tabHI", name="tabHI")

            def load_lo(lo_rows_ap):
                nc.scalar.dma_start(
                    tabLO[:].rearrange("p (k f) -> p k f", f=D),
                    lo_rows_ap.rearrange("(p k) f -> p k f", p=128))

            def load_hi(hi_rows_ap):
                nc.scalar.dma_start(
                    tabHI[:].rearrange("p (k f) -> p k f", f=D),
                    hi_rows_ap.rearrange("(p k) f -> p k f", p=128))

            # ---- DRAM state ----
            hE = [dram.tile([D, NP_], f16, name=f"hE{k}", tag=f"hE{k}")
                  for k in range(2)]
            hO = [dram.tile([D, NP_], f16, name=f"hO{k}", tag=f"hO{k}")
                  for k in range(2)]
            Tt = [dram.tile([NTAB, D], f16, name=f"T{k}", tag=f"T{k}")
                  for k in range(STEPS + 1)]
            def make_shared(rep):
                slo = [dram.tile([LO_CAP, D], f16, name=f"slo{rep}_{k}",
                                 tag=f"slo{rep}_{k}")
                       for k in range(STEPS + 1)]
                shi = [dram.tile([HI_CAP, D], f16, name=f"shi{rep}_{k}",
                                 tag=f"shi{rep}_{k}")
                       for k in range(STEPS + 1)]
                return slo, shi

            def zero_table(t):
                zr = zero_sb[:].rearrange("p (a f) -> p a f", f=D)
                r0 = 0
                while r0 < NTAB:
                    zc = 2048 if NTAB - r0 >= 2048 else NTAB - r0
                    nc.scalar.dma_start(
                        t[:][r0:r0 + zc, :].rearrange("(a p) f -> p a f", p=128),
                        zr[:, :zc // 128, :])
                    r0 += zc

            def half_rows(t, hi):
                return t[:][HI_BASE:NTAB, :] if hi else t[:][0:HI_BASE, :]

            def body(kind, seg, i, hin, hout, tgt):
                a, b = seg // 2, seg % 2
                off = meta["seg_start"][seg]
                gg = off // group + i          # global group index
                tab_b = tabHI if b else tabLO
                tab_a = tabHI if a else tabLO

                gi = gip.tile([128, 4 * group // 16], i16, tag="gi")
                nc.sync.dma_start(
                    gi[:], gIdx_t.ap()[:, bass.ds(gg * (4 * group // 16),
                                                  4 * group // 16)])
                AE = emb.tile([D, 1, group], f16, tag="AE")
                AO = emb.tile([D, 1, group], f16, tag="AO")
                nc.gpsimd.dma_gather(
                    AE[:], tab_b[:], gi[:, bass.ds(0, group // 16)],
                    num_idxs=group, num_idxs_reg=group, elem_size=D,
                    transpose=True, single_packet=False,
                    sbuf_tokens_per_rank=128, sbuf_free_dim_per_rank=256)
                nc.gpsimd.dma_gather(
                    AO[:], tab_a[:], gi[:, bass.ds(group // 16, group // 16)],
                    num_idxs=group, num_idxs_reg=group, elem_size=D,
                    transpose=True, single_packet=False,
                    sbuf_tokens_per_rank=128, sbuf_free_dim_per_rank=256)
                AEf = AE[:].rearrange("p a n -> p (a n)")
                AOf = AO[:].rearrange("p a n -> p (a n)")

                if kind == "init":
                    xE = work.tile([DE, group], f16, tag="xE")
                    nc.sync.dma_start(
                        xE[:], efE_t.ap()[:, bass.ds(off + i * group, group)])
                    xO = work.tile([DE, group], f16, tag="xO")
                    nc.sync.dma_start(
                        xO[:], efO_t.ap()[:, bass.ds(off + i * group, group)])
                else:
                    xE = work.tile([D, group], f16, tag="xE")
                    nc.sync.dma_start(
                        xE[:], hin[0][:][:, bass.ds(off + i * group, group)])
                    xO = work.tile([D, group], f16, tag="xO")
                    nc.sync.dma_start(
                        xO[:], hin[1][:][:, bass.ds(off + i * group, group)])
                    # msg = gathered - h_other (in place on the gathered tile)
                    nc.vector.tensor_sub(AEf, AEf, xO[:])
                    nc.vector.tensor_sub(AOf, AOf, xE[:])

                hts = []
                ems = []
                for arr in range(2):
                    pool = psA if arr == 0 else psB
                    msg = (AEf, AOf)[arr]
                    h_t = work.tile([D, group], f16, tag=f"ht{arr}")
                    em_t = emb.tile([128, GP, D], f16, tag=f"em{arr}")
                    for sub in range(GS):
                        c0 = sub * SLAB
                        ps_u = pool.tile([D, SLAB], f32, tag="u")
                        if kind == "init":
                            x_fm = (xE, xO)[arr]
                            nc.tensor.matmul(ps_u[:], WiA[:],
                                             msg[:, c0:c0 + SLAB],
                                             start=True, stop=False)
                            nc.tensor.matmul(ps_u[:], WiB[:],
                                             x_fm[:, c0:c0 + SLAB],
                                             start=False, stop=True)
                            bias = bi_sb
                        else:
                            h_self = (xE, xO)[arr]
                            nc.tensor.matmul(ps_u[:], Wu[:],
                                             msg[:, c0:c0 + SLAB],
                                             start=True, stop=False)
                            nc.tensor.matmul(ps_u[:], id16[:],
                                             h_self[:, c0:c0 + SLAB],
                                             start=False, stop=True)
                            bias = bu_sb
                        nc.scalar.activation(
                            h_t[:, c0:c0 + SLAB], ps_u[:],
                            mybir.ActivationFunctionType.Relu, bias=bias[:])
                        ps_e = pool.tile([128, NB, D], f16, tag="em")
                        for blk in range(NB):
                            nc.tensor.transpose(
                                ps_e[:, blk, :],
                                h_t[:, c0 + blk * 128:c0 + (blk + 1) * 128],
                                id16[:])
                        nc.scalar.copy(
                            em_t[:, bass.ds(sub * NB, NB), :], ps_e[:])
                    nc.sync.dma_start(
                        hout[arr][:][:, bass.ds(off + i * group, group)],
                        h_t[:])
                    hts.append(h_t)
                    ems.append(em_t)

                nc.gpsimd.dma_scatter_add(
                    half_rows(tgt, a), ems[0][:],
                    gi[:, bass.ds(2 * (group // 16), group // 16)],
                    num_idxs=group, num_idxs_reg=group, elem_size=D,
                    single_packet=True)
                nc.gpsimd.dma_scatter_add(
                    half_rows(tgt, b), ems[1][:],
                    gi[:, bass.ds(3 * (group // 16), group // 16)],
                    num_idxs=group, num_idxs_reg=group, elem_size=D,
                    single_packet=True)

            def run_seg(kind, seg, hin, hout, tgt):
                n_groups = meta["seg_sz"][seg] // group
                with tc.For_i(0, n_groups, unroll) as i:
                    for j in range(unroll):
                        body(kind, seg, i + j, hin, hout, tgt)

            def allreduce(src_ap, dst):
                nc.gpsimd.collective_compute(
                    "AllReduce", mybir.AluOpType.add,
                    replica_groups=[list(range(n_cores))],
                    ins=[src_ap], outs=[dst[:]])

            # ---- schedule ----
            for rep in range(REPEAT):
                if rep == 0:
                    for t in Tt:
                        zero_table(t)
                load_lo(nfp_t.ap()[0:HI_BASE, :])
                load_hi(nfp_t.ap()[HI_BASE:NTAB, :])
                SLO, SHI = make_shared(rep)
                for p in range(STEPS + 1):
                    kind = "init" if p == 0 else "step"
                    tgt = Tt[p]
                    hin = None if p == 0 else (hE[(p + 1) % 2], hO[(p + 1) % 2])
                    hout = (hE[p % 2], hO[p % 2])
                    if p >= 1:
                        load_lo(SLO[p - 1][:])
                    run_seg(kind, 0, hin, hout, tgt)
                    if p >= 1:
                        load_hi(SHI[p - 1][:])
                    run_seg(kind, 1, hin, hout, tgt)
                    run_seg(kind, 2, hin, hout, tgt)
                    # LO half is written by segs 0 (E+O), 1 (E), 2 (O); HI by
                    # segs 1 (O), 2 (E), 3 (E+O).  Each AllReduce must come
                    # after every writer of its half.
                    allreduce(half_rows(tgt, 0), SLO[p])
                    run_seg(kind, 3, hin, hout, tgt)
                    allreduce(half_rows(tgt, 1), SHI[p])

                # ---- final: out = relu([nf || agg] @ Wf + bf), token order --
                aggL, aggH = SLO[STEPS], SHI[STEPS]
                nf_lo = nfp_t.ap()[0:HI_BASE, :].rearrange(
                    "(p k) f -> p k f", p=128)
                nf_hi = nfp_t.ap()[HI_BASE:NTAB, :].rearrange(
                    "(p k) f -> p k f", p=128)
                ag_lo = aggL[:].rearrange("(p k) f -> p k f", p=128)
                ag_hi = aggH[:].rearrange("(p k) f -> p k f", p=128)
                o_lo = out_t.ap()[0:HI_BASE, :].rearrange(
                    "(p k) f -> p k f", p=128)
                o_hi = out_t.ap()[HI_BASE:NTAB, :].rearrange(
                    "(p k) f -> p k f", p=128)

                def final_body(j, nf_v, ag_v, o_v):
                    nf_em = emb.tile([128, NB, D], f16, tag="fnf")
                    nc.sync.dma_start(nf_em[:], nf_v[:, bass.ds(j * NB, NB), :])
                    ag_em = emb.tile([128, NB, D], f16, tag="fag")
                    nc.sync.dma_start(ag_em[:], ag_v[:, bass.ds(j * NB, NB), :])
                    ps_n = psA.tile([D, SLAB], f16, tag="em")
                    ps_g = psB.tile([D, SLAB], f16, tag="em")
                    for blk in range(NB):
                        nc.tensor.transpose(ps_n[:, blk * 128:(blk + 1) * 128],
                                            nf_em[:, blk, :], id16[:])
                        nc.tensor.transpose(ps_g[:, blk * 128:(blk + 1) * 128],
                                            ag_em[:, blk, :], id16[:])
                    nf_fm = work.tile([D, SLAB], f16, tag="fnm")
                    nc.scalar.copy(nf_fm[:], ps_n[:])
                    ag_fm = work.tile([D, SLAB], f16, tag="fgm")
                    nc.scalar.copy(ag_fm[:], ps_g[:])
                    ps_o = psA.tile([128, NB, D], f32, tag="u")
                    for blk in range(NB):
                        nc.tensor.matmul(ps_o[:, blk, :],
                                         nf_fm[:, blk * 128:(blk + 1) * 128],
                                         WfA[:], start=True, stop=False)
                        nc.tensor.matmul(ps_o[:, blk, :],
                                         ag_fm[:, blk * 128:(blk + 1) * 128],
                                         WfB[:], start=False, stop=True)
                    o_t = work.tile([128, NB, D], f32, tag="fot")
                    nc.vector.tensor_add(
                        o_t[:], ps_o[:],
                        bf4_sb[:].rearrange("p (a f) -> p a f", f=D))
                    nc.scalar.activation(o_t[:], o_t[:],
                                         mybir.ActivationFunctionType.Relu)
                    nc.sync.dma_start(o_v[:, bass.ds(j * NB, NB), :], o_t[:])

                with tc.For_i(0, T_LO // NB, 1) as j:
                    final_body(j, nf_lo, ag_lo, o_lo)
                with tc.For_i(0, T_HI // NB, 1) as j:
                    final_body(j, nf_hi, ag_hi, o_hi)

    nc.compile()
    return nc


LAST_RESULTS = None
LAST_TIMES = None


def _run_spmd(nc, in_maps, time_iters=0):
    """Execute the bass module on len(in_maps) axon cores via PJRT."""
    import time as _time

    import jax
    from jax.experimental.shard_map import shard_map
    from jax.sharding import Mesh, NamedSharding, PartitionSpec

    from concourse import bass2jax, mybir

    bass2jax.install_neuronx_cc_hook()
    n_cores = len(in_maps)
    partition_name = (nc.partition_id_tensor.name
                      if nc.partition_id_tensor else None)
    in_names, out_names, out_avals, zero_outs = [], [], [], []
    for alloc in nc.m.functions[0].allocations:
        if not isinstance(alloc, mybir.MemoryLocationSet):
            continue
        name = alloc.memorylocations[0].name
        if alloc.kind == "ExternalInput":
            if name != partition_name:
                in_names.append(name)
        elif alloc.kind == "ExternalOutput":
            shape = tuple(alloc.tensor_shape)
            dtype = mybir.dt.np(alloc.dtype)
            out_names.append(name)
            out_avals.append(jax.core.ShapedArray(shape, dtype))
            zero_outs.append(np.zeros(shape, dtype))
    n_params = len(in_names)
    full_in_names = list(in_names) + list(out_names)
    if partition_name is not None:
        full_in_names.append(partition_name)

    def _body(*args):
        operands = list(args)
        if partition_name is not None:
            operands.append(bass2jax.partition_id_tensor())
        outs = bass2jax._bass_exec_p.bind(
            *operands,
            out_avals=tuple(out_avals),
            in_names=tuple(full_in_names),
            out_names=tuple(out_names),
            lowering_input_output_aliases=(),
            sim_require_finite=True,
            sim_require_nnan=True,
            nc=nc,
        )
        return tuple(outs)

    devices = jax.devices()[:n_cores]
    mesh = Mesh(np.asarray(devices), ("core",))
    spec = NamedSharding(mesh, PartitionSpec("core"))
    n_in = n_params + len(zero_outs)
    fn = jax.jit(shard_map(_body, mesh=mesh,
                           in_specs=(PartitionSpec("core"),) * n_in,
                           out_specs=(PartitionSpec("core"),) * len(out_names),
                           check_rep=False))
    dev_in = [
        jax.device_put(
            np.concatenate([np.asarray(in_maps[c][k]) for c in range(n_cores)], 0),
            spec)
        for k in in_names
    ]
    dev_zero = [
        jax.device_put(np.zeros((n_cores * z.shape[0], *z.shape[1:]), z.dtype), spec)
        for z in zero_outs
    ]
    out = fn(*dev_in, *dev_zero)
    jax.block_until_ready(out)
    times = []
    for _ in range(time_iters):
        t0 = _time.perf_counter()
        out2 = fn(*dev_in, *dev_zero)
        jax.block_until_ready(out2)
        times.append(_time.perf_counter() - t0)
    results = [
        {name: np.asarray(out[i]).reshape(n_cores, *out_avals[i].shape)[c]
         for i, name in enumerate(out_names)}
        for c in range(n_cores)
    ]
    return results, times


def _common_inputs(Wi, bi, Wu, bu, Wf, bf):
    s = SCALE
    Wi = np.asarray(Wi, np.float32)
    Wu = np.asarray(Wu, np.float32)
    Wf = np.asarray(Wf, np.float32)
    bi = np.asarray(bi, np.float32)
    bu = np.asarray(bu, np.float32)
    bf = np.asarray(bf, np.float32)
    return {
        "WiA": (Wi[:D] * s).astype(np.float16),
        "WiB": (Wi[D:D + DE] * s).astype(np.float16),
        "Wu": Wu.astype(np.float16),
        "WfA": Wf[:D].astype(np.float16),
        "WfB": (Wf[D:2 * D] / s).astype(np.float16),
        "ident": np.eye(D, dtype=np.float16),
        "bi": (bi * s).reshape(D, 1),
        "bu": (bu * s).reshape(D, 1),
        "bf4": np.tile(bf, (D, SLAB // D)).reshape(D, SLAB),
    }


def kernel(node_feature, edge_feature, edge_src, edge_dst,
           Wi, bi, Wu, bu, Wf, bf):
    global LAST_RESULTS, LAST_TIMES
    shards, meta = _prep(node_feature, edge_feature, edge_src, edge_dst)
    nc = _build(meta)
    common = _common_inputs(Wi, bi, Wu, bu, Wf, bf)
    in_maps = [dict(sh, **common) for sh in shards]
    time_iters = int(os.environ.get("KERNEL_TIME_ITERS", "0"))
    results, times = _run_spmd(nc, in_maps, time_iters=time_iters)
    LAST_RESULTS = results
    LAST_TIMES = times
    o = np.asarray(results[0]["out"])
    N = meta["N"]
    return np.concatenate([o[0:HB], o[HI_BASE:HI_BASE + (N - HB)]], axis=0)



# revision 13
# speedup vs baseline: 1.2836x; 1.2836x over previous
"""DMPNN message-passing kernel for 8 Trainium2 NeuronCores (Bass/Tile), v2.

Strategy (edge/data parallel):
  - Edge pairs (2k, 2k+1) sharded across 8 cores; each core splits pairs into
    arrays hE/hO so rev(e) is the same row of the sibling array.
  - Everything fp16.  relu is positively homogeneous and the reference biases
    enter linearly, so h is computed at scale s=2^-12 (folded into Wi/bi/bu
    offline, un-folded via Wf) to stay in fp16 range.
  - The aggregate node table lives in SBUF (packed 256B tokens, partition =
    token & 127).  Gathers are SBUF-source dma_gather(transpose=True) which
    produce feature-major tiles directly: no HBM random reads, no PE
    transposes on the gather side.
  - segment_sum = fp16 dma_scatter_add into an HBM table (windowed so each
    instruction has distinct rows), then a 2-half fp16 AllReduce with Shared
    outputs, overlapped with compute of the independent half, then a bulk
    partition-contiguous reload of the SBUF table.
"""
import os
import sys

sys.path.insert(0, "/opt/trn_rl_repo")

import numpy as np

N_CORES = 8
D = 128
DE = 32
STEPS = 4
SLAB = 512
GROUP = 2048
UNROLL = 2
HB = 32256            # LO real-node boundary (63*512)
T_LO = 256            # free-dim token slots per partition, LO table
LO_CAP = T_LO * 128   # 32768
LO_TRASH = HB         # row-in-half used for padded slots
HI_BASE = LO_CAP
T_HI = 140
HI_CAP = T_HI * 128   # 17920
NTAB = HI_BASE + HI_CAP   # 50688
SCALE = np.float32(2.0 ** -12)


def _ceil(x, m):
    return (x + m - 1) // m * m


def _tok(r, t):
    """node-row-in-half -> SBUF token id (partition-contiguous bulk load)."""
    return (r % t) * 128 + r // t


def _window_assign(s, d, group, node_max, max_win=192):
    """Assign each pair to a window of size `group` such that within every
    window all d values are distinct and all s values are distinct (the
    dma_scatter_add engine-race constraint).  Greedy rounds, vectorized."""
    n = s.size
    win = np.full(n, -1, np.int32)
    used_s = np.zeros((node_max, max_win), bool)
    used_d = np.zeros((node_max, max_win), bool)
    full = np.zeros(max_win, bool)
    cnt = np.zeros(max_win, np.int64)
    rem = np.arange(n)
    while rem.size:
        free = ~(used_s[s[rem]] | used_d[d[rem]] | full[None, :])
        assert free.any(axis=1).all(), "window assigner ran out of windows"
        w = np.argmax(free, axis=1).astype(np.int64)
        order = np.lexsort((rem, w))
        ws, rs = w[order], rem[order]
        ds_, ss_ = d[rs], s[rs]
        kd = ws * np.int64(node_max) + ds_
        ks = ws * np.int64(node_max) + ss_
        first_d = np.zeros(ws.size, bool)
        first_s = np.zeros(ws.size, bool)
        od = np.lexsort((np.arange(ws.size), kd))
        first_d[od[np.concatenate(([True], kd[od][1:] != kd[od][:-1]))]] = True
        os_ = np.lexsort((np.arange(ws.size), ks))
        first_s[os_[np.concatenate(([True], ks[os_][1:] != ks[os_][:-1]))]] = True
        uw, st, cts = np.unique(ws, return_index=True, return_counts=True)
        rank = np.arange(ws.size) - np.repeat(st, cts)
        ok = first_d & first_s & (rank < np.repeat(group - cnt[uw], cts))
        acc = rs[ok]
        wacc = ws[ok]
        win[acc] = wacc
        used_d[d[acc], wacc] = True
        used_s[s[acc], wacc] = True
        np.add.at(cnt, wacc, 1)
        full = cnt >= group
        rem = rem[win[rem] < 0]
    return win, (int(cnt.nonzero()[0].max()) + 1) if n else 0


def _prep(node_feature, edge_feature, edge_src, edge_dst,
          n_cores=N_CORES, group=GROUP, unroll=UNROLL,
          hb=HB, t_lo=T_LO, t_hi=T_HI):
    LO_CAP = t_lo * 128
    HI_BASE = LO_CAP
    HI_CAP = t_hi * 128
    NTAB = HI_BASE + HI_CAP
    HB_, T_LO_, T_HI_, LO_TRASH_ = hb, t_lo, t_hi, hb
    node_feature = np.asarray(node_feature, np.float32)
    edge_feature = np.asarray(edge_feature, np.float16)
    edge_src = np.asarray(edge_src)
    edge_dst = np.asarray(edge_dst)
    N = node_feature.shape[0]
    E = edge_src.shape[0]
    P = E // 2
    assert P % n_cores == 0
    per = P // n_cores
    assert N <= hb + HI_CAP - 1

    s_all = edge_src[0::2].astype(np.int64)
    d_all = edge_dst[0::2].astype(np.int64)
    efE_all = edge_feature[0::2]
    efO_all = edge_feature[1::2]

    HI_TRASH = N - hb      # row-in-half for HI trash

    # padded node-feature table, fp16, trash/pad rows zero
    nfp = np.zeros((NTAB, D), np.float16)
    nfp[0:hb] = node_feature[0:hb].astype(np.float16)
    nfp[HI_BASE:HI_BASE + (N - hb)] = node_feature[hb:N].astype(np.float16)

    cores = []
    nwin = np.zeros((n_cores, 4), np.int64)
    for c in range(n_cores):
        sl = slice(c * per, (c + 1) * per)
        sc, dc = s_all[sl], d_all[sl]
        a = (dc >= hb).astype(np.int64)
        b = (sc >= hb).astype(np.int64)
        seg = a * 2 + b
        per_seg = []
        for g in range(4):
            m = np.flatnonzero(seg == g)
            s_m, d_m = sc[m], dc[m]
            degd = np.bincount(d_m, minlength=N)
            degs = np.bincount(s_m, minlength=N)
            prio = np.argsort(-(degd[d_m] + degs[s_m]), kind="stable")
            win_p, nw = _window_assign(s_m[prio], d_m[prio], group, N)
            win = np.empty_like(win_p)
            win[prio] = win_p
            key = np.lexsort((s_m, win))
            per_seg.append((m[key], win[key], nw))
            nwin[c, g] = nw
        cores.append((sc, dc, efE_all[sl], efO_all[sl], per_seg))
    gchunk = group * unroll
    seg_nw = [int(_ceil(max(int(nwin[:, g].max()), 1) * group, gchunk)) // group
              for g in range(4)]
    seg_sz = [nw * group for nw in seg_nw]
    NP_ = int(sum(seg_sz))
    seg_start = [0, seg_sz[0], seg_sz[0] + seg_sz[1],
                 seg_sz[0] + seg_sz[1] + seg_sz[2]]

    def wrap16(v):
        t = v.astype(np.int16).reshape(-1, 16).T       # [16, n/16]
        return np.ascontiguousarray(np.tile(t, (8, 1)))  # [128, n/16]

    shards = []
    for c in range(n_cores):
        sc, dc, efE_c, efO_c, per_seg = cores[c]
        sRow = np.zeros(NP_, np.int64)   # scatter row within half b
        dRow = np.zeros(NP_, np.int64)   # scatter row within half a
        sTok = np.zeros(NP_, np.int64)   # gather token within half b
        dTok = np.zeros(NP_, np.int64)   # gather token within half a
        efE_p = np.zeros((NP_, DE), np.float16)
        efO_p = np.zeros((NP_, DE), np.float16)
        for g in range(4):
            a, b = g // 2, g % 2
            st = seg_start[g]
            t_a, t_b = (t_hi if a else t_lo), (t_hi if b else t_lo)
            tr_a = HI_TRASH if a else LO_TRASH_
            tr_b = HI_TRASH if b else LO_TRASH_
            sRow[st:st + seg_sz[g]] = tr_b
            dRow[st:st + seg_sz[g]] = tr_a
            sTok[st:st + seg_sz[g]] = _tok(tr_b, t_b)
            dTok[st:st + seg_sz[g]] = _tok(tr_a, t_a)
            order, wins, nw = per_seg[g]
            if order.size:
                counts = np.bincount(wins, minlength=nw)
                assert counts.max() <= group
                starts = np.concatenate(([0], np.cumsum(counts)))[:-1]
                rank = np.arange(order.size) - starts[wins]
                pos = st + wins * group + rank
                sr = sc[order] - hb * b
                dr = dc[order] - hb * a
                sRow[pos] = sr
                dRow[pos] = dr
                sTok[pos] = _tok(sr, t_b)
                dTok[pos] = _tok(dr, t_a)
                efE_p[pos] = efE_c[order]
                efO_p[pos] = efO_c[order]
        assert sRow.max() < 32768 and dRow.max() < 32768
        assert sTok.max() < 32768 and dTok.max() < 32768

        # idx stream: per group g: [sTok | dTok | dRow | sRow]
        gidx = np.empty((NP_ // group, 4, group), np.int64)
        for g in range(NP_ // group):
            sl = slice(g * group, (g + 1) * group)
            gidx[g, 0] = sTok[sl]
            gidx[g, 1] = dTok[sl]
            gidx[g, 2] = dRow[sl]
            gidx[g, 3] = sRow[sl]
        shards.append({
            "nfp": nfp,
            "efE": np.ascontiguousarray(efE_p.T),
            "efO": np.ascontiguousarray(efO_p.T),
            "gIdx": wrap16(gidx.reshape(-1)),
        })

    meta = dict(N=N, NP=NP_, seg_sz=seg_sz, seg_start=seg_start,
                n_cores=n_cores, group=group, unroll=unroll,
                HB=hb, T_LO=t_lo, T_HI=t_hi, LO_CAP=LO_CAP,
                HI_BASE=HI_BASE, HI_CAP=HI_CAP, NTAB=NTAB)
    return shards, meta


def _build(meta):
    import concourse.bass as bass
    import concourse.tile as tile
    from concourse import bacc, mybir

    f32 = mybir.dt.float32
    f16 = mybir.dt.float16
    i16 = mybir.dt.int16
    NP_ = meta["NP"]
    group = meta["group"]
    unroll = meta["unroll"]
    n_cores = meta["n_cores"]
    T_LO = meta["T_LO"]
    T_HI = meta["T_HI"]
    LO_CAP = meta["LO_CAP"]
    HI_BASE = meta["HI_BASE"]
    HI_CAP = meta["HI_CAP"]
    NTAB = meta["NTAB"]
    REPEAT = int(os.environ.get("KERNEL_REPEAT", "1"))

    nc = bacc.Bacc("TRN2", target_bir_lowering=False, debug=False,
                   enable_asserts=False, num_devices=n_cores,
                   num_swdge_queues=4)

    nfp_t = nc.dram_tensor("nfp", [NTAB, D], f16, kind="ExternalInput")
    efE_t = nc.dram_tensor("efE", [DE, NP_], f16, kind="ExternalInput")
    efO_t = nc.dram_tensor("efO", [DE, NP_], f16, kind="ExternalInput")
    gIdx_t = nc.dram_tensor("gIdx", [128, 4 * NP_ // 16], i16, kind="ExternalInput")
    WiA_t = nc.dram_tensor("WiA", [D, D], f16, kind="ExternalInput")
    WiB_t = nc.dram_tensor("WiB", [DE, D], f16, kind="ExternalInput")
    Wu_t = nc.dram_tensor("Wu", [D, D], f16, kind="ExternalInput")
    WfA_t = nc.dram_tensor("WfA", [D, D], f16, kind="ExternalInput")
    WfB_t = nc.dram_tensor("WfB", [D, D], f16, kind="ExternalInput")
    id_t = nc.dram_tensor("ident", [D, D], f16, kind="ExternalInput")
    bi_t = nc.dram_tensor("bi", [D, 1], f32, kind="ExternalInput")
    bu_t = nc.dram_tensor("bu", [D, 1], f32, kind="ExternalInput")
    bf4_t = nc.dram_tensor("bf4", [D, SLAB], f32, kind="ExternalInput")
    out_t = nc.dram_tensor("out", [NTAB, D], f32, kind="ExternalOutput")
    GP = group // 128
    GS = group // SLAB
    NB = SLAB // 128

    with tile.TileContext(nc) as tc:
        with (
            tc.tile_pool(name="const", bufs=1) as constp,
            tc.tile_pool(name="tabp", bufs=1) as tabp,
            tc.tile_pool(name="gip", bufs=2) as gip,
            tc.tile_pool(name="work", bufs=2) as work,
            tc.tile_pool(name="emb", bufs=2) as emb,
            tc.tile_pool(name="psA", bufs=2, space="PSUM") as psA,
            tc.tile_pool(name="psB", bufs=2, space="PSUM") as psB,
            tc.tile_pool(name="dram", bufs=1, space="DRAM") as dram,
        ):
            # ---- constants ----
            def cload(name, shape, dt_, src):
                t = constp.tile(shape, dt_, tag=name, name=name)
                nc.sync.dma_start(t[:], src)
                return t

            WiA = cload("WiA", [D, D], f16, WiA_t.ap())
            WiB = cload("WiB", [DE, D], f16, WiB_t.ap())
            Wu = cload("Wu", [D, D], f16, Wu_t.ap())
            WfA = cload("WfA", [D, D], f16, WfA_t.ap())
            WfB = cload("WfB", [D, D], f16, WfB_t.ap())
            id16 = cload("id16", [D, D], f16, id_t.ap())
            bi_sb = cload("bi", [D, 1], f32, bi_t.ap())
            bu_sb = cload("bu", [D, 1], f32, bu_t.ap())
            bf4_sb = cload("bf4", [D, SLAB], f32, bf4_t.ap())
            zero_sb = constp.tile([128, 2048], f16, tag="zero", name="zero")
            nc.vector.memset(zero_sb[:], 0.0)

            # ---- SBUF node tables ----
            tabLO = tabp.tile([128, T_LO * D], f16, tag="tabLO", name="tabLO")
            tabHI = tabp.tile([128, T_HI * D], f16, tag="tabHI", name="tabHI")

            def load_lo(lo_rows_ap):
                nc.scalar.dma_start(
                    tabLO[:].rearrange("p (k f) -> p k f", f=D),
                    lo_rows_ap.rearrange("(p k) f -> p k f", p=128))

            def load_hi(hi_rows_ap):
                nc.scalar.dma_start(
                    tabHI[:].rearrange("p (k f) -> p k f", f=D),
                    hi_rows_ap.rearrange("(p k) f -> p k f", p=128))

            # ---- DRAM state ----
            hE = [dram.tile([D, NP_], f16, name=f"hE{k}", tag=f"hE{k}")
                  for k in range(2)]
            hO = [dram.tile([D, NP_], f16, name=f"hO{k}", tag=f"hO{k}")
                  for k in range(2)]
            Tt = [dram.tile([NTAB, D], f16, name=f"T{k}", tag=f"T{k}")
                  for k in range(STEPS + 1)]
            def make_shared(rep):
                slo = [dram.tile([LO_CAP, D], f16, name=f"slo{rep}_{k}",
                                 tag=f"slo{rep}_{k}")
                       for k in range(STEPS + 1)]
                shi = [dram.tile([HI_CAP, D], f16, name=f"shi{rep}_{k}",
                                 tag=f"shi{rep}_{k}")
                       for k in range(STEPS + 1)]
                return slo, shi

            def zero_table(t):
                zr = zero_sb[:].rearrange("p (a f) -> p a f", f=D)
                r0 = 0
                while r0 < NTAB:
                    zc = 2048 if NTAB - r0 >= 2048 else NTAB - r0
                    nc.scalar.dma_start(
                        t[:][r0:r0 + zc, :].rearrange("(a p) f -> p a f", p=128),
                        zr[:, :zc // 128, :])
                    r0 += zc

            def half_rows(t, hi):
                return t[:][HI_BASE:NTAB, :] if hi else t[:][0:HI_BASE, :]

            def body(kind, seg, i, hin, hout, tgt):
                a, b = seg // 2, seg % 2
                off = meta["seg_start"][seg]
                gg = off // group + i          # global group index
                tab_b = tabHI if b else tabLO
                tab_a = tabHI if a else tabLO

                gi = gip.tile([128, 4 * group // 16], i16, tag="gi")
                nc.sync.dma_start(
                    gi[:], gIdx_t.ap()[:, bass.ds(gg * (4 * group // 16),
                                                  4 * group // 16)])
                AE = emb.tile([D, 1, group], f16, tag="AE")
                AO = emb.tile([D, 1, group], f16, tag="AO")
                nc.gpsimd.dma_gather(
                    AE[:], tab_b[:], gi[:, bass.ds(0, group // 16)],
                    num_idxs=group, num_idxs_reg=group, elem_size=D,
                    transpose=True, single_packet=False,
                    sbuf_tokens_per_rank=128, sbuf_free_dim_per_rank=256)
                nc.gpsimd.dma_gather(
                    AO[:], tab_a[:], gi[:, bass.ds(group // 16, group // 16)],
                    num_idxs=group, num_idxs_reg=group, elem_size=D,
                    transpose=True, single_packet=False,
                    sbuf_tokens_per_rank=128, sbuf_free_dim_per_rank=256)
                AEf = AE[:].rearrange("p a n -> p (a n)")
                AOf = AO[:].rearrange("p a n -> p (a n)")

                if kind == "init":
                    xE = work.tile([DE, group], f16, tag="xE")
                    nc.sync.dma_start(
                        xE[:], efE_t.ap()[:, bass.ds(off + i * group, group)])
                    xO = work.tile([DE, group], f16, tag="xO")
                    nc.sync.dma_start(
                        xO[:], efO_t.ap()[:, bass.ds(off + i * group, group)])
                else:
                    xE = work.tile([D, group], f16, tag="xE")
                    nc.sync.dma_start(
                        xE[:], hin[0][:][:, bass.ds(off + i * group, group)])
                    xO = work.tile([D, group], f16, tag="xO")
                    nc.sync.dma_start(
                        xO[:], hin[1][:][:, bass.ds(off + i * group, group)])
                    # msg = gathered - h_other (in place on the gathered tile)
                    nc.vector.tensor_sub(AEf, AEf, xO[:])
                    nc.vector.tensor_sub(AOf, AOf, xE[:])

                hts = []
                ems = []
                for arr in range(2):
                    pool = psA if arr == 0 else psB
                    msg = (AEf, AOf)[arr]
                    h_t = work.tile([D, group], f16, tag=f"ht{arr}")
                    em_t = emb.tile([128, GP, D], f16, tag=f"em{arr}")
                    for sub in range(GS):
                        c0 = sub * SLAB
                        ps_u = pool.tile([D, SLAB], f32, tag="u")
                        if kind == "init":
                            x_fm = (xE, xO)[arr]
                            nc.tensor.matmul(ps_u[:], WiA[:],
                                             msg[:, c0:c0 + SLAB],
                                             start=True, stop=False)
                            nc.tensor.matmul(ps_u[:], WiB[:],
                                             x_fm[:, c0:c0 + SLAB],
                                             start=False, stop=True)
                            bias = bi_sb
                        else:
                            h_self = (xE, xO)[arr]
                            nc.tensor.matmul(ps_u[:], Wu[:],
                                             msg[:, c0:c0 + SLAB],
                                             start=True, stop=False)
                            nc.tensor.matmul(ps_u[:], id16[:],
                                             h_self[:, c0:c0 + SLAB],
                                             start=False, stop=True)
                            bias = bu_sb
                        nc.scalar.activation(
                            h_t[:, c0:c0 + SLAB], ps_u[:],
                            mybir.ActivationFunctionType.Relu, bias=bias[:])
                        ps_e = pool.tile([128, NB, D], f16, tag="em")
                        for blk in range(NB):
                            nc.tensor.transpose(
                                ps_e[:, blk, :],
                                h_t[:, c0 + blk * 128:c0 + (blk + 1) * 128],
                                id16[:])
                        nc.scalar.copy(
                            em_t[:, bass.ds(sub * NB, NB), :], ps_e[:])
                    nc.sync.dma_start(
                        hout[arr][:][:, bass.ds(off + i * group, group)],
                        h_t[:])
                    hts.append(h_t)
                    ems.append(em_t)

                nc.gpsimd.dma_scatter_add(
                    half_rows(tgt, a), ems[0][:],
                    gi[:, bass.ds(2 * (group // 16), group // 16)],
                    num_idxs=group, num_idxs_reg=group, elem_size=D,
                    single_packet=True)
                nc.gpsimd.dma_scatter_add(
                    half_rows(tgt, b), ems[1][:],
                    gi[:, bass.ds(3 * (group // 16), group // 16)],
                    num_idxs=group, num_idxs_reg=group, elem_size=D,
                    single_packet=True)

            def run_seg(kind, seg, hin, hout, tgt):
                n_groups = meta["seg_sz"][seg] // group
                with tc.For_i(0, n_groups, unroll) as i:
                    for j in range(unroll):
                        body(kind, seg, i + j, hin, hout, tgt)

            def allreduce(src_ap, dst):
                nc.gpsimd.collective_compute(
                    "AllReduce", mybir.AluOpType.add,
                    replica_groups=[list(range(n_cores))],
                    ins=[src_ap], outs=[dst[:]])

            # ---- schedule ----
            for rep in range(REPEAT):
                if rep == 0:
                    for t in Tt:
                        zero_table(t)
                load_lo(nfp_t.ap()[0:HI_BASE, :])
                load_hi(nfp_t.ap()[HI_BASE:NTAB, :])
                SLO, SHI = make_shared(rep)
                for p in range(STEPS + 1):
                    kind = "init" if p == 0 else "step"
                    tgt = Tt[p]
                    hin = None if p == 0 else (hE[(p + 1) % 2], hO[(p + 1) % 2])
                    hout = (hE[p % 2], hO[p % 2])
                    if p >= 1:
                        load_lo(SLO[p - 1][:])
                    run_seg(kind, 0, hin, hout, tgt)
                    if p >= 1:
                        load_hi(SHI[p - 1][:])
                    run_seg(kind, 1, hin, hout, tgt)
                    run_seg(kind, 2, hin, hout, tgt)
                    # LO half is written by segs 0 (E+O), 1 (E), 2 (O); HI by
                    # segs 1 (O), 2 (E), 3 (E+O).  Each AllReduce must come
                    # after every writer of its half.
                    allreduce(half_rows(tgt, 0), SLO[p])
                    run_seg(kind, 3, hin, hout, tgt)
                    allreduce(half_rows(tgt, 1), SHI[p])

                # ---- final: out = relu([nf || agg] @ Wf + bf), token order --
                aggL, aggH = SLO[STEPS], SHI[STEPS]
                nf_lo = nfp_t.ap()[0:HI_BASE, :].rearrange(
                    "(p k) f -> p k f", p=128)
                nf_hi = nfp_t.ap()[HI_BASE:NTAB, :].rearrange(
                    "(p k) f -> p k f", p=128)
                ag_lo = aggL[:].rearrange("(p k) f -> p k f", p=128)
                ag_hi = aggH[:].rearrange("(p k) f -> p k f", p=128)
                o_lo = out_t.ap()[0:HI_BASE, :].rearrange(
                    "(p k) f -> p k f", p=128)
                o_hi = out_t.ap()[HI_BASE:NTAB, :].rearrange(
                    "(p k) f -> p k f", p=128)

                def final_body(j, nf_v, ag_v, o_v):
                    nf_em = emb.tile([128, NB, D], f16, tag="fnf")
                    nc.sync.dma_start(nf_em[:], nf_v[:, bass.ds(j * NB, NB), :])
                    ag_em = emb.tile([128, NB, D], f16, tag="fag")
                    nc.sync.dma_start(ag_em[:], ag_v[:, bass.ds(j * NB, NB), :])
                    ps_n = psA.tile([D, SLAB], f16, tag="em")
                    ps_g = psB.tile([D, SLAB], f16, tag="em")
                    for blk in range(NB):
                        nc.tensor.transpose(ps_n[:, blk * 128:(blk + 1) * 128],
                                            nf_em[:, blk, :], id16[:])
                        nc.tensor.transpose(ps_g[:, blk * 128:(blk + 1) * 128],
                                            ag_em[:, blk, :], id16[:])
                    nf_fm = work.tile([D, SLAB], f16, tag="fnm")
                    nc.scalar.copy(nf_fm[:], ps_n[:])
                    ag_fm = work.tile([D, SLAB], f16, tag="fgm")
                    nc.scalar.copy(ag_fm[:], ps_g[:])
                    ps_o = psA.tile([128, NB, D], f32, tag="u")
                    for blk in range(NB):
                        nc.tensor.matmul(ps_o[:, blk, :],
                                         nf_fm[:, blk * 128:(blk + 1) * 128],
                                         WfA[:], start=True, stop=False)
                        nc.tensor.matmul(ps_o[:, blk, :],
                                         ag_fm[:, blk * 128:(blk + 1) * 128],
                                         WfB[:], start=False, stop=True)
                    o_t = work.tile([128, NB, D], f32, tag="fot")
                    nc.vector.tensor_add(
                        o_t[:], ps_o[:],
                        bf4_sb[:].rearrange("p (a f) -> p a f", f=D))
                    nc.scalar.activation(o_t[:], o_t[:],
                                         mybir.ActivationFunctionType.Relu)
                    nc.sync.dma_start(o_v[:, bass.ds(j * NB, NB), :], o_t[:])

                with tc.For_i(0, T_LO // NB, 1) as j:
                    final_body(j, nf_lo, ag_lo, o_lo)
                with tc.For_i(0, T_HI // NB, 1) as j:
                    final_body(j, nf_hi, ag_hi, o_hi)

    nc.compile()
    return nc


LAST_RESULTS = None
LAST_TIMES = None


def _run_spmd(nc, in_maps, time_iters=0):
    """Execute the bass module on len(in_maps) axon cores via PJRT."""
    import time as _time

    import jax
    from jax.experimental.shard_map import shard_map
    from jax.sharding import Mesh, NamedSharding, PartitionSpec

    from concourse import bass2jax, mybir

    bass2jax.install_neuronx_cc_hook()
    n_cores = len(in_maps)
    partition_name = (nc.partition_id_tensor.name
                      if nc.partition_id_tensor else None)
    in_names, out_names, out_avals, zero_outs = [], [], [], []
    for alloc in nc.m.functions[0].allocations:
        if not isinstance(alloc, mybir.MemoryLocationSet):
            continue
        name = alloc.memorylocations[0].name
        if alloc.kind == "ExternalInput":
            if name != partition_name:
                in_names.append(name)
        elif alloc.kind == "ExternalOutput":
            shape = tuple(alloc.tensor_shape)
            dtype = mybir.dt.np(alloc.dtype)
            out_names.append(name)
            out_avals.append(jax.core.ShapedArray(shape, dtype))
            zero_outs.append(np.zeros(shape, dtype))
    n_params = len(in_names)
    full_in_names = list(in_names) + list(out_names)
    if partition_name is not None:
        full_in_names.append(partition_name)

    def _body(*args):
        operands = list(args)
        if partition_name is not None:
            operands.append(bass2jax.partition_id_tensor())
        outs = bass2jax._bass_exec_p.bind(
            *operands,
            out_avals=tuple(out_avals),
            in_names=tuple(full_in_names),
            out_names=tuple(out_names),
            lowering_input_output_aliases=(),
            sim_require_finite=True,
            sim_require_nnan=True,
            nc=nc,
        )
        return tuple(outs)

    devices = jax.devices()[:n_cores]
    mesh = Mesh(np.asarray(devices), ("core",))
    spec = NamedSharding(mesh, PartitionSpec("core"))
    n_in = n_params + len(zero_outs)
    fn = jax.jit(shard_map(_body, mesh=mesh,
                           in_specs=(PartitionSpec("core"),) * n_in,
                           out_specs=(PartitionSpec("core"),) * len(out_names),
                           check_rep=False))
    dev_in = [
        jax.device_put(
            np.concatenate([np.asarray(in_maps[c][k]) for c in range(n_cores)], 0),
            spec)
        for k in in_names
    ]
    dev_zero = [
        jax.device_put(np.zeros((n_cores * z.shape[0], *z.shape[1:]), z.dtype), spec)
        for z in zero_outs
    ]
    out = fn(*dev_in, *dev_zero)
    jax.block_until_ready(out)
    times = []
    for _ in range(time_iters):
        t0 = _time.perf_counter()
        out2 = fn(*dev_in, *dev_zero)
        jax.block_until_ready(out2)
        times.append(_time.perf_counter() - t0)
    results = [
        {name: np.asarray(out[i]).reshape(n_cores, *out_avals[i].shape)[c]
         for i, name in enumerate(out_names)}
        for c in range(n_cores)
    ]
    return results, times


def _common_inputs(Wi, bi, Wu, bu, Wf, bf):
    s = SCALE
    Wi = np.asarray(Wi, np.float32)
    Wu = np.asarray(Wu, np.float32)
    Wf = np.asarray(Wf, np.float32)
    bi = np.asarray(bi, np.float32)
    bu = np.asarray(bu, np.float32)
    bf = np.asarray(bf, np.float32)
    return {
        "WiA": (Wi[:D] * s).astype(np.float16),
        "WiB": (Wi[D:D + DE] * s).astype(np.float16),
        "Wu": Wu.astype(np.float16),
        "WfA": Wf[:D].astype(np.float16),
        "WfB": (Wf[D:2 * D] / s).astype(np.float16),
        "ident": np.eye(D, dtype=np.float16),
        "bi": (bi * s).reshape(D, 1),
        "bu": (bu * s).reshape(D, 1),
        "bf4": np.tile(bf, (D, SLAB // D)).reshape(D, SLAB),
    }


def kernel(node_feature, edge_feature, edge_src, edge_dst,
           Wi, bi, Wu, bu, Wf, bf):
    global LAST_RESULTS, LAST_TIMES
    shards, meta = _prep(node_feature, edge_feature, edge_src, edge_dst)
    nc = _build(meta)
    common = _common_inputs(Wi, bi, Wu, bu, Wf, bf)
    in_maps = [dict(sh, **common) for sh in shards]
    time_iters = int(os.environ.get("KERNEL_TIME_ITERS", "0"))
    results, times = _run_spmd(nc, in_maps, time_iters=time_iters)
    LAST_RESULTS = results
    LAST_TIMES = times
    o = np.asarray(results[0]["out"])
    N = meta["N"]
    return np.concatenate([o[0:HB], o[HI_BASE:HI_BASE + (N - HB)]], axis=0)

